# revision 26
# baseline (speedup 1.0000x reference)
"""Beta-TCVAE modified-KLD loss on 8 Trainium2 NeuronCores (Bass/Tile).

Math
----
reference computes, per row i (n=2048, zdim=32):
  lqx[i]  = sum_k logN(z[i,k]; mu[i,k], e^{lv[i,k]})
  lpz[i]  = sum_k logN(z[i,k]; 0, 1)
  lqz[i]  = lse_j( logiw[i,j] + s[i,j] ),          s[i,j] = sum_k pre[i,j,k]
  lqpm[i] = sum_k lse_j( logiw[i,j] + pre[i,j,k] )
  loss    = lqx + 5*lqz - 5*lqpm - lpz            (= mi + 6*tc + dwkl)
with pre[i,j,k] = -0.5*(log2pi + lv[j,k] + (z[i,k]-mu[j,k])^2 * e^{-lv[j,k]}).

logiw (stratified importance weights) is column-constant: c2[j] =
log(1/m) for j>=2, c2[0]=log(1/n_ds), c2[1]=log(strat), with ONE special
entry logiw[m-1, 0] = log(strat).  (The reference's `Wf[::m+1]` stride
equals the row length, so it fills column 0, not the diagonal.)  The
special entry is fixed up on the host from the device-returned raw sums.

Per-element exponent expands to a quadratic in z:
  2*(pre + c2) = -( z^2*iv + z*(-2*mu*iv) + (mu^2*iv + lv + log2pi - 2*c2) )
so each [i, j] block of exponents is a rank-3 product realized as a
masked contract-96 matmul:
  rhs  stack3[96, 2048] = [iv^T; b^T; g^T]        (b = -2*mu*iv)
  lhsT zb[96, k, i]     = diag-masked [z^2; z; 1]  (only rows k, 32+k,
                          64+k are nonzero for dimension k — matmul
                          operands must start at partition 0/32/64, so
                          per-k partition slicing is not possible)
  V_k = zb[:,k,:].T @ stack3   -> PSUM [128 i, 2048 j]
  ACT: Exp(-0.5 * V_k) with accum_out -> P1[i, k] = sum_j exp-term
(no max-subtraction: exponents stay far inside fp32 exp range; the
underflowing tail is negligible vs each row's max term)

The row-lse numerator P2[i] = sum_j exp(c2[j] + s[i,j]) uses one
contract-65 matmul per i-half ([z2^T; z^T; 1] x [iv^T; b^T; G2]) kept in
full fp32 (G2 carries O(100) magnitudes; fp32r truncation there would be
fatal).  The P1 matmuls use float32r, which streams 4x faster than fp32
(1 vs 4 cycles/row) at tf32-like operand precision — verified loss
rel-err ~4e-4.

Ln of P2 is evaluated as Ln(P2 * 2^44) - 44*ln2: P2 sits around 1e-19..
1e-22, below the Ln spline's input domain (~2^-60 edge clamps).

Sharding: data-parallel over query rows i (256 per core); every core
holds the full column batch (the [96, 2048] operand stack).  Host-side
work is O(n*zdim) marshalling only (transposes, exp/mul operand prep,
the c2 vector from dataset_size); all O(n^2) work runs on device, ACT
exp-bound at ~64 x 2us per core.
"""

import os
from contextlib import ExitStack

import numpy as np

import concourse.bacc as bacc
import concourse.mybir as mybir
import concourse.tile as tile
from concourse import bass_utils

N = 2048
ZD = 32
NCORES = 8
RB = N // NCORES  # 256 rows per core
IH = RB // 128  # 2 partition halves per core
L2PI = float(np.log(2.0 * np.pi))
F32 = mybir.dt.float32
F32R = mybir.dt.float32r
AL = mybir.AluOpType
AF = mybir.ActivationFunctionType
LNSHIFT = 44  # Ln(P2 * 2^LNSHIFT) - LNSHIFT*ln2

_NC = None
LAST_RESULT = None  # BassKernelResults of the most recent run (for test.py)


def _build_kernel_body(nc, tc, io, ctx):
    singles = ctx.enter_context(tc.tile_pool(name="singles", bufs=1))
    psum = ctx.enter_context(tc.tile_pool(name="psum", bufs=2, space="PSUM"))
    escr = ctx.enter_context(tc.tile_pool(name="escr", bufs=2))

    # ---- load host-marshalled operands -------------------------------
    stack3 = singles.tile([3 * ZD, N], F32R, tag="stack3")
    ZB = singles.tile([3 * ZD, ZD, RB], F32R, tag="ZB")
    BLK = singles.tile([2 * ZD + 1, N], F32, tag="BLK")
    ZS = singles.tile([2 * ZD + 1, RB], F32, tag="ZS")
    zI = singles.tile([128, IH, ZD], F32, tag="zI")
    muI = singles.tile([128, IH, ZD], F32, tag="muI")
    lvI = singles.tile([128, IH, ZD], F32, tag="lvI")
    ivI = singles.tile([128, IH, ZD], F32, tag="ivI")
    nc.sync.dma_start(out=stack3, in_=io["stack3"])
    nc.sync.dma_start(out=ZB, in_=io["zb"])
    nc.sync.dma_start(out=BLK, in_=io["blk"])
    nc.sync.dma_start(out=ZS, in_=io["zs"])
    nc.sync.dma_start(out=zI, in_=io["zI"].rearrange("(h p) k -> p h k", p=128))
    nc.sync.dma_start(out=muI, in_=io["muI"].rearrange("(h p) k -> p h k", p=128))
    nc.sync.dma_start(out=lvI, in_=io["lvI"].rearrange("(h p) k -> p h k", p=128))
    nc.sync.dma_start(out=ivI, in_=io["ivI"].rearrange("(h p) k -> p h k", p=128))

    # ---- diagonal terms: lqx, lpz ------------------------------------
    dI = singles.tile([128, IH, ZD], F32, tag="dI")
    scr = singles.tile([128, ZD], F32, tag="scr")
    comps = [
        singles.tile([128, 4], F32, tag=f"comps{h}", name=f"comps{h}")
        for h in range(IH)
    ]
    acc = singles.tile([128, 4], F32, tag="acc")
    nc.vector.tensor_sub(out=dI, in0=zI, in1=muI)
    nc.vector.tensor_mul(out=dI, in0=dI, in1=dI)
    for h in range(IH):
        # q1 = sum_k d^2 * iv
        nc.vector.scalar_tensor_tensor(
            out=scr,
            in0=dI[:, h, :],
            scalar=0.0,
            in1=ivI[:, h, :],
            op0=AL.bypass,
            op1=AL.mult,
            accum_out=acc[:, 0:1],
        )
        nc.vector.tensor_reduce(
            out=acc[:, 1:2], in_=lvI[:, h, :], axis=mybir.AxisListType.X, op=AL.add
        )
        # zs = sum_k z^2
        nc.vector.scalar_tensor_tensor(
            out=scr,
            in0=zI[:, h, :],
            scalar=0.0,
            in1=zI[:, h, :],
            op0=AL.bypass,
            op1=AL.mult,
            accum_out=acc[:, 2:3],
        )
        # lqx = -0.5*(32*l2pi + lvsum + q1)
        nc.vector.scalar_tensor_tensor(
            out=acc[:, 3:4],
            in0=acc[:, 1:2],
            scalar=ZD * L2PI,
            in1=acc[:, 0:1],
            op0=AL.add,
            op1=AL.add,
        )
        nc.vector.tensor_scalar_mul(
            out=comps[h][:, 0:1], in0=acc[:, 3:4], scalar1=-0.5
        )
        # lpz = -0.5*(32*l2pi + zs)
        nc.vector.tensor_scalar(
            out=comps[h][:, 3:4],
            in0=acc[:, 2:3],
            scalar1=ZD * L2PI,
            op0=AL.add,
            scalar2=-0.5,
            op1=AL.mult,
        )

    # ---- main loop: P1[i,k] = sum_j exp(-0.5 * V_k) ------------------
    # shared tiles across halves: the h=0 epilogue then depends (at tile
    # granularity) on h=1 writes too, keeping Ln after all Exp ops to
    # avoid ACT table-set thrash
    P1all = singles.tile([128, IH, ZD], F32, tag="P1all")
    P2all = singles.tile([128, IH], F32, tag="P2all")
    P1 = [P1all[:, h, :] for h in range(IH)]
    P2 = [P2all[:, h : h + 1] for h in range(IH)]
    for h in range(IH):
        lh = slice(h * 128, (h + 1) * 128)
        for k in range(ZD):
            V = psum.tile([128, N], F32, tag="V")
            for c in range(4):
                cs = slice(c * 512, (c + 1) * 512)
                nc.tensor.matmul(
                    V[:, cs], ZB[:, k, lh], stack3[:, cs], start=True, stop=True
                )
            E = escr.tile([128, N], F32, tag="E")
            nc.scalar.activation(
                out=E, in_=V, func=AF.Exp, scale=-0.5, accum_out=P1[h][:, k : k + 1]
            )
        # row-lse numerator: P2 = sum_j exp(-0.5 * V2), contract 65, fp32
        V = psum.tile([128, N], F32, tag="V")
        for c in range(4):
            cs = slice(c * 512, (c + 1) * 512)
            nc.tensor.matmul(V[:, cs], ZS[:, lh], BLK[:, cs], start=True, stop=True)
        E = escr.tile([128, N], F32, tag="E")
        nc.scalar.activation(
            out=E, in_=V, func=AF.Exp, scale=-0.5, accum_out=P2[h]
        )

    # ---- logs + loss assembly ----------------------------------------
    for h in range(IH):
        lnscr = singles.tile([128, ZD], F32, tag="lnscr")
        nc.scalar.activation(
            out=lnscr, in_=P1[h], func=AF.Ln, accum_out=comps[h][:, 2:3]
        )
        nc.scalar.activation(
            out=comps[h][:, 1:2],
            in_=P2[h],
            func=AF.Ln,
            scale=float(2.0**LNSHIFT),
        )
        nc.vector.tensor_scalar_add(
            out=comps[h][:, 1:2],
            in0=comps[h][:, 1:2],
            scalar1=float(-LNSHIFT * np.log(2.0)),
        )
        ta = singles.tile([128, 2], F32, tag="ta")
        loss_t = singles.tile([128, 1], F32, tag=f"loss{h}", name=f"loss{h}")
        nc.vector.tensor_sub(
            out=ta[:, 0:1], in0=comps[h][:, 1:2], in1=comps[h][:, 2:3]
        )
        nc.vector.tensor_sub(
            out=ta[:, 1:2], in0=comps[h][:, 0:1], in1=comps[h][:, 3:4]
        )
        nc.vector.scalar_tensor_tensor(
            out=loss_t,
            in0=ta[:, 0:1],
            scalar=5.0,
            in1=ta[:, 1:2],
            op0=AL.mult,
            op1=AL.add,
        )
        rs = slice(h * 128, (h + 1) * 128)
        nc.sync.dma_start(out=io["loss"][rs, :], in_=loss_t)
        nc.sync.dma_start(out=io["comps"][rs, :], in_=comps[h])
        nc.sync.dma_start(out=io["p1"][rs, :], in_=P1[h])
        nc.sync.dma_start(out=io["p2"][rs, :], in_=P2[h])


def _build():
    # Bacc (not raw Bass): its finalize() runs generate_event_semaphores,
    # which splits multi-sem waits — TRN2 instructions carry at most one.
    nc = bacc.Bacc(
        "TRN2", target_bir_lowering=False, debug=False, num_devices=NCORES
    )
    io = {}
    for name, shape, dt in [
        ("stack3", [3 * ZD, N], F32R),
        ("zb", [3 * ZD, ZD, RB], F32R),
        ("blk", [2 * ZD + 1, N], F32),
        ("zs", [2 * ZD + 1, RB], F32),
        ("zI", [RB, ZD], F32),
        ("muI", [RB, ZD], F32),
        ("lvI", [RB, ZD], F32),
        ("ivI", [RB, ZD], F32),
    ]:
        io[name] = nc.dram_tensor(name, shape, dt, kind="ExternalInput").ap()
    for name, shape in [
        ("loss", [RB, 1]),
        ("comps", [RB, 4]),
        ("p1", [RB, ZD]),
        ("p2", [RB, 1]),
    ]:
        io[name] = nc.dram_tensor(name, shape, F32, kind="ExternalOutput").ap()
    with tile.TileContext(nc) as tc:
        with ExitStack() as ctx:
            _build_kernel_body(nc, tc, io, ctx)
    nc.finalize()
    return nc


def kernel(z, mu, logvar, dataset_size):
    global _NC, LAST_RESULT
    z = np.ascontiguousarray(np.asarray(z, dtype=np.float32))
    mu = np.ascontiguousarray(np.asarray(mu, dtype=np.float32))
    lv = np.ascontiguousarray(np.asarray(logvar, dtype=np.float32))
    nds = int(np.asarray(dataset_size).reshape(-1)[0])
    m = N - 1
    strat = (nds - m) / (nds * m)
    c2w = np.full(N, 1.0 / m, np.float64)
    c2w[0] = 1.0 / nds
    c2w[1] = strat
    c2 = np.log(c2w)  # [N] f64

    if _NC is None:
        _NC = _build()

    # O(n*zdim) operand marshalling (f64 for accuracy, stored f32):
    lv64 = lv.astype(np.float64)
    mu64 = mu.astype(np.float64)
    iv = np.exp(-lv64)
    b = -2.0 * mu64 * iv
    g = mu64 * mu64 * iv + lv64 + L2PI - 2.0 * c2[:, None]
    G2 = (mu64 * mu64 * iv + lv64).sum(axis=1) + ZD * L2PI - 2.0 * c2
    stack3 = np.concatenate([iv.T, b.T, g.T], axis=0).astype(np.float32)
    blk = np.concatenate([iv.T, b.T, G2[None, :]], axis=0).astype(np.float32)

    kk = np.arange(ZD)
    in_maps = []
    for c in range(NCORES):
        sl = slice(c * RB, (c + 1) * RB)
        zt = z[sl].T.astype(np.float64)
        zb = np.zeros((3 * ZD, ZD, RB), np.float32)
        zb[kk, kk, :] = (zt * zt).astype(np.float32)
        zb[ZD + kk, kk, :] = zt.astype(np.float32)
        zb[2 * ZD + kk, kk, :] = 1.0
        zs = np.concatenate([zt * zt, zt, np.ones((1, RB))], axis=0).astype(
            np.float32
        )
        in_maps.append(
            {
                "stack3": stack3,
                "zb": zb,
                "blk": blk,
                "zs": zs,
                "zI": z[sl].copy(),
                "muI": mu[sl].copy(),
                "lvI": lv[sl].copy(),
                "ivI": iv[sl].astype(np.float32),
            }
        )
    trace = bool(os.environ.get("KBENCH_TRACE"))
    res = bass_utils.run_bass_kernel_spmd(
        _NC,
        in_maps,
        core_ids=list(range(NCORES)),
        trace=trace,
        trace_cores=[0] if trace else None,
    )
    LAST_RESULT = res
    loss = np.concatenate([r["loss"][:, 0] for r in res.results])
    comps = np.concatenate([r["comps"] for r in res.results])
    p1 = np.concatenate([r["p1"] for r in res.results])
    p2 = np.concatenate([r["p2"][:, 0] for r in res.results])

    # host fixup for the single non-column-constant logiw entry (m-1, 0)
    r = m - 1
    pre0 = -0.5 * (
        L2PI + lv64[0] + (z[r].astype(np.float64) - mu64[0]) ** 2 * iv[0]
    )
    dw = strat - 1.0 / nds
    P1r = p1[r].astype(np.float64) + dw * np.exp(pre0)
    P2r = float(p2[r]) + dw * np.exp(pre0.sum())
    lqpm_r = float(np.log(P1r).sum())
    lqz_r = float(np.log(P2r))
    comps[r, 1] = lqz_r
    comps[r, 2] = lqpm_r
    loss[r] = comps[r, 0] + 5.0 * lqz_r - 5.0 * lqpm_r - comps[r, 3]

    c64 = comps.astype(np.float64)
    mi = (c64[:, 0] - c64[:, 1]).mean()
    tc_ = (c64[:, 1] - c64[:, 2]).mean()
    dwkl = (c64[:, 2] - c64[:, 3]).mean()
    return (
        loss.astype(np.float32),
        np.float32(mi),
        np.float32(tc_),
        np.float32(dwkl),
    )
